# revision 36
# baseline (speedup 1.0000x reference)
"""Trainium2 Bass kernel for a 6-layer post-LN transformer encoder.

Sharding: data-parallel over batch — B=8, one batch element per NeuronCore,
no collectives.  Each core runs the full 6-layer encoder on its [S, D] slice.

Device-side layout: activations are kept feature-major ([D, S], "xT") in SBUF
so that every matmul can use the natural input-major weights as the stationary
(lhsT) operand and PE contracts over the partition dim:

  out[m, n] = sum_k lhsT[k, m] * rhs[k, n]

Attention is computed transposed (scoresT[t, s]) so softmax needs no
transposes: denominators fall out of a ones-column in the ctx matmul, and the
per-column 1/denom broadcast is a k=1 matmul on PE.

v2 changes vs baseline:
  * bf16 activations + weights (fp32 PSUM accumulate).  2x DVE throughput on
    SBUF elementwise ops, half the DMA traffic.  Small stats rows stay fp32r.
  * Score matmuls for the two heads of a pair are emitted adjacently: K=64
    row-group tiling (auto tile_position (0,0)/(64,0)) runs them concurrently.
  * Both heads' scores land in one [P,1024] PSUM tile -> a single Exp
    activation per (pair, nh, t) halves ACT instruction overhead.
  * ctx for head B uses an M=128 stationary [ones|0(63)|V_B] so its rows land
    at partitions 64..127 (denominator at row 0): the per-pair ctx tile is
    [128, S] and Wo contracts K=128 (half the Wo matmuls).
  * Software-pipelined emission: scores(t+1) ahead of ctx(t), FFN1(ft) ahead
    of FFN2(ft-1), QK of pair p+1 between the two nh halves of pair p.
  * relu / PSUM->SBUF broadcast copies split between ScalarE and DVE.
"""

import numpy as np

L, H, D, DK, DFF = 6, 8, 512, 64, 2048
B, S = 8, 1024
EPS = 1e-5
P = 128
NDT = D // P        # 4  d-tiles
NST = S // P        # 8  s/t-tiles
NFT = DFF // P      # 16 dff-tiles
NPAIR = H // 2      # 4  head pairs
NH = S // 512       # 2  n-halves (512-wide fp32 matmul free dim)
FCH = 2             # W1 streamed in chunks of 2 dff-tiles
VBLK = 196          # per-pair V block: [V_A(64)|1|.|1|zeros|V_B(64) @130]
SCALE = 1.0 / np.sqrt(np.float32(DK))

_CACHE = {}


def _bf16():
    from concourse import mybir

    return mybir.dt.np(mybir.dt.bfloat16)


def _build_nc():
    import concourse.bass as bass
    import concourse.bacc as bacc
    import concourse.tile as tile
    from concourse import mybir

    fp32 = mybir.dt.float32
    fp32r = mybir.dt.float32r
    bf16 = mybir.dt.bfloat16
    AF = mybir.ActivationFunctionType
    OP = mybir.AluOpType

    class _Bacc(bacc.Bacc):
        # Exp (softmax) and Ln (layernorm rstd) live in different default
        # activation-table sets, causing ~50 table-load thrashes (~2.7us
        # each). Restrict both to natural_log_exp_and_others (which holds
        # both) so one load serves the whole kernel. Positional set ids are
        # preserved; only the function->set resolution changes.
        def insert_act_table_loads(self):
            from concourse.hw_specs import get_activation_tables
            import bass_rust as _bass_rust

            has_act = any(
                isinstance(i, mybir.InstActivation)
                for b in self.main_func.blocks
                for i in b.instructions
            )
            if not has_act:
                return
            AF2 = mybir.ActivationFunctionType
            tables = []
            for name, fns in get_activation_tables(self.m.arch).items():
                if name != "natural_log_exp_and_others":
                    fns = fns - {AF2.Exp, AF2.Ln}
                tables.append((name, fns))
            _bass_rust.insert_act_table_loads(self, tables)

    nc = _Bacc()

    from concourse.hw_specs import get_activation_tables

    _nl_set = get_activation_tables(nc.m.arch).get(
        "natural_log_exp_and_others", set()
    )
    relu_on_act = AF.Relu in _nl_set and AF.Copy in _nl_set
    copy_on_act = AF.Copy in _nl_set

    def mm(out, lhsT, rhs, **kw):
        return nc.tensor.matmul(out, lhsT, rhs, **kw)

    def mmr(out, lhsT, rhs, **kw):
        # fp32r matmul for the small stats/broadcast rows
        return nc.tensor.matmul(
            out, lhsT.bitcast(fp32r), rhs.bitcast(fp32r), **kw
        )

    def f(ap):
        # view a float32r tile as plain fp32 for DVE/ACT reads
        return ap.bitcast(fp32)

    x_d = nc.declare_dram_parameter("x", [NDT, P, S], bf16, isOutput=False)
    wq_d = nc.declare_dram_parameter(
        "wq", [L, P, NDT, NPAIR, P], bf16, isOutput=False
    )
    wk_d = nc.declare_dram_parameter(
        "wk", [L, P, NDT, NPAIR, P], bf16, isOutput=False
    )
    wv_d = nc.declare_dram_parameter("wv", [L, P, NDT, H * DK], bf16, isOutput=False)
    wo_d = nc.declare_dram_parameter(
        "wo", [L, P, NPAIR, NDT, P], bf16, isOutput=False
    )
    w1_d = nc.declare_dram_parameter(
        "w1", [L, P, NDT, NFT, P], bf16, isOutput=False
    )
    w2_d = nc.declare_dram_parameter(
        "w2", [L, P, NFT, NDT, P], bf16, isOutput=False
    )
    g1_d = nc.declare_dram_parameter("g1", [L, P, NDT], fp32, isOutput=False)
    be1_d = nc.declare_dram_parameter("be1", [L, P, NDT], fp32, isOutput=False)
    g2_d = nc.declare_dram_parameter("g2", [L, P, NDT], fp32, isOutput=False)
    be2_d = nc.declare_dram_parameter("be2", [L, P, NDT], fp32, isOutput=False)
    b1_d = nc.declare_dram_parameter("b1", [L, P, NFT], fp32, isOutput=False)
    b2_d = nc.declare_dram_parameter("b2", [L, P, NDT], fp32, isOutput=False)
    ones_d = nc.declare_dram_parameter("ones", [P, P], fp32r, isOutput=False)
    out_d = nc.declare_dram_parameter("out", [NDT, P, S], fp32, isOutput=True)

    with tile.TileContext(nc) as tc:
        from contextlib import ExitStack

        with ExitStack() as ctx:
            ec = ctx.enter_context
            ec(
                nc.allow_low_precision(
                    reason="bf16 matmul operands; fp32 PSUM accumulation"
                )
            )
            # --- SBUF pools ---
            const_p = ec(tc.tile_pool(name="const", bufs=1))
            wts_p = ec(tc.tile_pool(name="wts", bufs=2))
            xt_p = ec(tc.tile_pool(name="xt", bufs=4))
            qk_p = ec(tc.tile_pool(name="qk", bufs=2))
            v_p = ec(tc.tile_pool(name="v", bufs=8))
            exp_p = ec(tc.tile_pool(name="exp", bufs=4))
            ctx_p = ec(tc.tile_pool(name="ctxp", bufs=4))
            mha_p = ec(tc.tile_pool(name="mha", bufs=4))
            ff1_p = ec(tc.tile_pool(name="ff1", bufs=4))
            ysq_p = ec(tc.tile_pool(name="ysq", bufs=2))
            bcs_p = ec(tc.tile_pool(name="bcs", bufs=2))
            rows_p = ec(tc.tile_pool(name="rows", bufs=1))
            out_p = ec(tc.tile_pool(name="outp", bufs=4))
            # --- PSUM pools: 4 + 2 + 2 = 8 banks ---
            # sc:  [P,1024] scores (A|B) tiles; FFN borrows [P,512] slots
            # acc: attention ctx accumulators (A, B)
            # mm:  short-lived matmul outputs (QKV/V/Wo/FFN1/LN stats+bc)
            pp_sc = ec(tc.tile_pool(name="pp_sc", bufs=2, space="PSUM"))
            pp_acc = ec(tc.tile_pool(name="pp_acc", bufs=2, space="PSUM"))
            pp_mm = ec(tc.tile_pool(name="pp_mm", bufs=2, space="PSUM"))

            # ones come from DRAM so the fp32r data counts as pre-rounded
            ones_full = const_p.tile([P, P], fp32r)
            nc.sync.dma_start(out=ones_full, in_=ones_d[:, :])
            ones_col_bf = const_p.tile([P, 1], bf16)
            nc.vector.memset(ones_col_bf, 1.0)
            # selector rows for the paired 1/denom broadcast:
            #   selA row: [1]*64 + [0]*64   selB row: [0]*64 + [1]*64
            selA_t = const_p.tile([P, P], fp32r)
            nc.vector.memset(f(selA_t)[:, 0:64], 1.0)
            nc.vector.memset(f(selA_t)[:, 64:128], 0.0)
            selB_t = const_p.tile([P, P], fp32r)
            nc.vector.memset(f(selB_t)[:, 0:64], 0.0)
            nc.vector.memset(f(selB_t)[:, 64:128], 1.0)
            zero_col = const_p.tile([P, 1], fp32)
            nc.vector.memset(zero_col, 0.0)
            eps_col = const_p.tile([P, 1], fp32)
            nc.vector.memset(eps_col, float(EPS))

            # layer-0 input
            xt = []
            for dt in range(NDT):
                t = xt_p.tile([P, S], bf16, tag="xt")
                nc.sync.dma_start(out=t, in_=x_d[dt])
                xt.append(t)

            def make_qk(pr, w_t, tag):
                # both nh halves' matmuls adjacent per dt: the second reuses
                # the stationary (no LDWEIGHTS reload on HW)
                dst = qk_p.tile([P, S], bf16, tag=tag, name="qkdst")
                ps0 = pp_mm.tile([P, 512], fp32, tag="mm", name="qkps0")
                ps1 = pp_mm.tile([P, 512], fp32, tag="mm", name="qkps1")
                for dt in range(NDT):
                    for nh, ps in ((0, ps0), (1, ps1)):
                        mm(
                            ps,
                            w_t[:, dt, pr, :],
                            xt[dt][:, nh * 512 : (nh + 1) * 512],
                            start=(dt == 0),
                            stop=(dt == NDT - 1),
                        )
                nc.vector.tensor_copy(dst[:, 0:512], ps0)
                nc.vector.tensor_copy(dst[:, 512:1024], ps1)
                return dst

            for l in range(L):
                # ---------------- weight loads (bufs=2 pools: next layer's
                # loads overlap this layer's compute) ------------------------
                wq_t = wts_p.tile([P, NDT, NPAIR, P], bf16, tag="wq")
                nc.sync.dma_start(out=wq_t, in_=wq_d[l])
                wk_t = wts_p.tile([P, NDT, NPAIR, P], bf16, tag="wk")
                nc.sync.dma_start(out=wk_t, in_=wk_d[l])
                wv_t = wts_p.tile([P, NDT, H * DK], bf16, tag="wv")
                nc.sync.dma_start(out=wv_t, in_=wv_d[l])
                wo_t = wts_p.tile([P, NPAIR, NDT, P], bf16, tag="wo")
                nc.sync.dma_start(out=wo_t, in_=wo_d[l])
                g1_t = wts_p.tile([P, NDT], fp32, tag="g1")
                nc.sync.dma_start(out=g1_t, in_=g1_d[l])
                be1_t = wts_p.tile([P, NDT], fp32, tag="be1")
                nc.sync.dma_start(out=be1_t, in_=be1_d[l])
                g2_t = wts_p.tile([P, NDT], fp32, tag="g2")
                nc.sync.dma_start(out=g2_t, in_=g2_d[l])
                be2_t = wts_p.tile([P, NDT], fp32, tag="be2")
                nc.sync.dma_start(out=be2_t, in_=be2_d[l])
                b1_t = wts_p.tile([P, NFT], fp32, tag="b1")
                nc.sync.dma_start(out=b1_t, in_=b1_d[l])
                b2_t = wts_p.tile([P, NDT], fp32, tag="b2")
                nc.sync.dma_start(out=b2_t, in_=b2_d[l])
                w1_t = wts_p.tile([P, NDT, NFT, P], bf16, tag="w1")
                nc.sync.dma_start(out=w1_t, in_=w1_d[l])
                w2_t = wts_p.tile([P, NFT, NDT, P], bf16, tag="w2")
                nc.sync.dma_start(out=w2_t, in_=w2_d[l])

                # ---------------- Q/K for pair 0 ----------------------------
                qt = [None] * NPAIR
                kt = [None] * NPAIR
                qt[0] = make_qk(0, wq_t, "qt")
                kt[0] = make_qk(0, wk_t, "kt")

                # ---------------- V = x @ Wv, packed per head pair ----------
                # vt[:, pr, 0:64]    = V of head 2*pr       (ctx rows 0..63)
                # vt[:, pr, 64]      = 1                    (denom A, row 64)
                # vt[:, pr, 66]      = 1                    (denom B, row 0)
                # vt[:, pr, 67:130]  = 0                    (junk rows 1..63)
                # vt[:, pr, 130:194] = V of head 2*pr+1     (ctx rows 64..127)
                v_tiles = [None] * NST

                def make_v(st):
                    vt = v_p.tile([P, NPAIR, VBLK], bf16, tag="v", name="vt")
                    ps = pp_mm.tile([P, 512], fp32, tag="mm", name="vps")
                    for dt in range(NDT):
                        mm(
                            ps,
                            xt[dt][:, st * P : (st + 1) * P],
                            wv_t[:, dt, :],
                            start=(dt == 0),
                            stop=(dt == NDT - 1),
                        )
                    psh = ps.rearrange("p (h k) -> p h k", h=H)
                    nc.vector.tensor_copy(vt[:, :, 0:DK], psh[:, 0::2, :])
                    nc.vector.tensor_copy(vt[:, :, 130 : 130 + DK], psh[:, 1::2, :])
                    nc.gpsimd.memset(vt[:, :, 67:130], 0.0)
                    nc.gpsimd.memset(vt[:, :, 64:67], 1.0)
                    v_tiles[st] = vt

                # V[0..1] up front; the rest fill just-in-time inside the
                # first pair's t-loop so exp work starts sooner
                make_v(0)
                make_v(1)

                # ---------------- attention ---------------------------------
                ctx_tiles = []
                for pr in range(NPAIR):
                    ch = ctx_p.tile([P, S], bf16, tag="ctx", name="ch")
                    ctx_tiles.append(ch)

                def scores(pr, nh, t):
                    ssl = slice(nh * 512, (nh + 1) * 512)
                    tsl = slice(t * P, (t + 1) * P)
                    sc = pp_sc.tile([P, 1024], fp32, tag="sc", name="sc")
                    # two K=64 matmuls on distinct row groups -> concurrent
                    mm(sc[:, 0:512], kt[pr][0:64, tsl], qt[pr][0:64, ssl])
                    mm(sc[:, 512:1024], kt[pr][64:128, tsl], qt[pr][64:128, ssl])
                    return sc

                def attend(pr, nh, vfill=False):
                    ssl = slice(nh * 512, (nh + 1) * 512)
                    psA = pp_acc.tile([P, 512], fp32, tag="acc", name="psA")
                    psB = pp_acc.tile([P, 512], fp32, tag="acc", name="psB")
                    sc_cur = scores(pr, nh, 0)
                    for t in range(NST):
                        if vfill and t + 2 < NST:
                            make_v(t + 2)
                        sc_next = scores(pr, nh, t + 1) if t + 1 < NST else None
                        e = exp_p.tile([P, 1024], bf16, tag="exp", name="e")
                        nc.scalar.activation(
                            e, sc_cur, AF.Exp, bias=zero_col, scale=float(SCALE)
                        )
                        vt = v_tiles[t]
                        mm(
                            psA[0:65, :],
                            vt[:, pr, 0 : DK + 1],
                            e[:, 0:512],
                            start=(t == 0),
                            stop=(t == NST - 1),
                        )
                        mm(
                            psB,
                            vt[:, pr, 66:194],
                            e[:, 512:1024],
                            start=(t == 0),
                            stop=(t == NST - 1),
                        )
                        sc_cur = sc_next
                    # normalize: ctx rows / denominator (A: row 64, B: row 0).
                    # ScalarE copies the raw ctx rows out first so the acc
                    # PSUM banks release early (next pair's ctx can start);
                    # the 1/denom scale then runs in-place in SBUF with the
                    # broadcast read straight from PSUM.
                    rA = rows_p.tile([65, 512], fp32r, tag="rA", bufs=2, name="rA")
                    nc.vector.reciprocal(rA[64:65], psA[64:65])
                    rB = rows_p.tile([1, 512], fp32r, tag="rB", bufs=2, name="rB")
                    nc.vector.reciprocal(rB, psB[0:1])
                    ch = ctx_tiles[pr]
                    nc.vector.tensor_copy(ch[0:64, ssl], psA[0:64])
                    nc.vector.tensor_copy(ch[64:128, ssl], psB[64:128])
                    # bc rows 0..63 = 1/dA, rows 64..127 = 1/dB via selector
                    # rows (two accumulating M=128 matmuls, no col tiling)
                    bc = pp_mm.tile([P, 512], fp32, tag="mm", name="bc")
                    mmr(bc, selA_t[64:65, :], rA[64:65], start=True, stop=False)
                    mmr(bc, selB_t[0:1, :], rB, start=False, stop=True)
                    nc.vector.tensor_mul(ch[0:64, ssl], ch[0:64, ssl], bc[0:64])
                    nc.vector.tensor_mul(ch[64:128, ssl], ch[64:128, ssl], bc[64:128])

                for pr in range(NPAIR):
                    attend(pr, 0, vfill=(pr == 0))
                    if pr + 1 < NPAIR:
                        # PE work to cover the normalize tail / acc release
                        qt[pr + 1] = make_qk(pr + 1, wq_t, "qt")
                        kt[pr + 1] = make_qk(pr + 1, wk_t, "kt")
                    attend(pr, 1)

                # ---------------- Wo + residual -> y (pre-LN1) --------------
                y = []
                for mt in range(NDT):
                    yt = mha_p.tile([P, S], bf16, tag="mha", name="yt")
                    y.append(yt)
                for mt in range(NDT):
                    ps0 = pp_mm.tile([P, 512], fp32, tag="mm", name="wops0")
                    ps1 = pp_mm.tile([P, 512], fp32, tag="mm", name="wops1")
                    for pr in range(NPAIR):
                        for nh, ps in ((0, ps0), (1, ps1)):
                            mm(
                                ps,
                                wo_t[:, pr, mt, :],
                                ctx_tiles[pr][:, nh * 512 : (nh + 1) * 512],
                                start=(pr == 0),
                                stop=(pr == NPAIR - 1),
                            )
                    nc.vector.tensor_add(y[mt][:, 0:512], ps0, xt[mt][:, 0:512])
                    nc.vector.tensor_add(y[mt][:, 512:1024], ps1, xt[mt][:, 512:1024])

                def ln_stats(yl, nh, bcm_sb, bcr_sb):
                    # per-half LN stats chain: ones-matmul sums -> mean/var
                    # rows -> rstd via Ln/Exp -> k=1 broadcast matmuls ->
                    # SBUF copies into the given [P,1024] broadcast tiles
                    ssl = slice(nh * 512, (nh + 1) * 512)
                    pool, tg = (pp_mm, "mm") if nh == 0 else (pp_sc, "sc")
                    p1 = pool.tile([1, 512], fp32, tag=tg, name="p1")
                    for dt in range(NDT):
                        mm(
                            p1,
                            ones_col_bf,
                            yl[dt][:, ssl],
                            start=(dt == 0),
                            stop=(dt == NDT - 1),
                        )
                    p2 = pool.tile([1, 512], fp32, tag=tg, name="p2")
                    for dt in range(NDT):
                        sq = ysq_p.tile([P, 512], bf16, tag="ysq", name="sq")
                        nc.vector.tensor_mul(sq, yl[dt][:, ssl], yl[dt][:, ssl])
                        mm(
                            p2,
                            ones_col_bf,
                            sq,
                            start=(dt == 0),
                            stop=(dt == NDT - 1),
                        )
                    mean = rows_p.tile(
                        [1, 512], fp32r, tag=f"mean{nh}", bufs=1, name="mean"
                    )
                    nc.vector.tensor_scalar_mul(mean, p1, 1.0 / D)
                    # mean^2 on ScalarE (Square with scale=1/D), in parallel
                    # with DVE computing the mean row
                    var = rows_p.tile(
                        [1, 512], fp32, tag=f"var{nh}", bufs=1, name="var"
                    )
                    if copy_on_act:
                        nc.scalar.activation(var, p1, AF.Square, scale=1.0 / D)
                    else:
                        nc.vector.tensor_mul(var, f(mean), f(mean))
                    nc.vector.scalar_tensor_tensor(
                        var, p2, 1.0 / D, var, OP.mult, OP.subtract
                    )
                    # rstd = exp(-0.5 * ln(var + eps))
                    nc.scalar.activation(var, var, AF.Ln, bias=eps_col[0:1])
                    rstd = rows_p.tile(
                        [1, 512], fp32r, tag=f"rstd{nh}", bufs=1, name="rstd"
                    )
                    nc.scalar.activation(
                        rstd, var, AF.Exp, bias=zero_col[0:1], scale=-0.5
                    )
                    bcm = pool.tile([P, 512], fp32, tag=tg, name="bcm")
                    mmr(bcm, ones_full[0:1, :], mean)
                    bcr = pool.tile([P, 512], fp32, tag=tg, name="bcr")
                    mmr(bcr, ones_full[0:1, :], rstd)
                    if copy_on_act:
                        nc.scalar.copy(bcm_sb[:, ssl], bcm)
                        nc.scalar.copy(bcr_sb[:, ssl], bcr)
                    else:
                        nc.vector.tensor_copy(bcm_sb[:, ssl], bcm)
                        nc.vector.tensor_copy(bcr_sb[:, ssl], bcr)

                def ln_norm(yl, g_t, be_t, bcm_sb, bcr_sb, out_tiles=None):
                    for dt in range(NDT):
                        for nh in range(NH):
                            ssl = slice(nh * 512, (nh + 1) * 512)
                            yr = yl[dt][:, ssl]
                            dst = (
                                yr
                                if out_tiles is None
                                else out_tiles[dt][:, ssl]
                            )
                            eng = (
                                nc.gpsimd
                                if dt >= 2 and out_tiles is None
                                else nc.vector
                            )
                            eng.tensor_sub(yr, yr, bcm_sb[:, ssl])
                            eng.tensor_mul(yr, yr, bcr_sb[:, ssl])
                            eng.tensor_scalar(
                                out=dst,
                                in0=yr,
                                scalar1=g_t[:, dt : dt + 1],
                                scalar2=be_t[:, dt : dt + 1],
                                op0=OP.mult,
                                op1=OP.add,
                            )

                def layernorm(yl, g_t, be_t, out_tiles=None):
                    # the two nh-half chains are interleaved stage by stage so
                    # DVE/ACT latency pipelines across halves
                    bcm_sb = bcs_p.tile([P, 1024], bf16, tag="bcs", name="bcmsb")
                    bcr_sb = bcs_p.tile([P, 1024], bf16, tag="bcs", name="bcrsb")
                    p1l, p2l, meanl, varl, rstdl = [], [], [], [], []
                    for nh in range(NH):
                        ssl = slice(nh * 512, (nh + 1) * 512)
                        pool, tg = (pp_mm, "mm") if nh == 0 else (pp_sc, "sc")
                        p1 = pool.tile([1, 512], fp32, tag=tg, name="p1")
                        for dt in range(NDT):
                            mm(
                                p1,
                                ones_col_bf,
                                yl[dt][:, ssl],
                                start=(dt == 0),
                                stop=(dt == NDT - 1),
                            )
                        p2 = pool.tile([1, 512], fp32, tag=tg, name="p2")
                        for dt in range(NDT):
                            sq = ysq_p.tile([P, 512], bf16, tag="ysq", name="sq")
                            nc.vector.tensor_mul(
                                sq, yl[dt][:, ssl], yl[dt][:, ssl]
                            )
                            mm(
                                p2,
                                ones_col_bf,
                                sq,
                                start=(dt == 0),
                                stop=(dt == NDT - 1),
                            )
                        p1l.append(p1)
                        p2l.append(p2)
                    for nh in range(NH):
                        mean = rows_p.tile(
                            [1, 512], fp32r, tag=f"mean{nh}", bufs=1, name="mean"
                        )
                        nc.vector.tensor_scalar_mul(mean, p1l[nh], 1.0 / D)
                        meanl.append(mean)
                    for nh in range(NH):
                        # mean^2 on ScalarE (Square with scale=1/D), parallel
                        # with the DVE mean rows
                        var = rows_p.tile(
                            [1, 512], fp32, tag=f"var{nh}", bufs=1, name="var"
                        )
                        if copy_on_act:
                            nc.scalar.activation(
                                var, p1l[nh], AF.Square, scale=1.0 / D
                            )
                        else:
                            nc.vector.tensor_mul(var, f(meanl[nh]), f(meanl[nh]))
                        nc.vector.scalar_tensor_tensor(
                            var, p2l[nh], 1.0 / D, var, OP.mult, OP.subtract
                        )
                        varl.append(var)
                    for nh in range(NH):
                        nc.scalar.activation(
                            varl[nh], varl[nh], AF.Ln, bias=eps_col[0:1]
                        )
                    for nh in range(NH):
                        rstd = rows_p.tile(
                            [1, 512], fp32r, tag=f"rstd{nh}", bufs=1, name="rstd"
                        )
                        nc.scalar.activation(
                            rstd, varl[nh], AF.Exp, bias=zero_col[0:1], scale=-0.5
                        )
                        rstdl.append(rstd)
                    for nh in range(NH):
                        ssl = slice(nh * 512, (nh + 1) * 512)
                        pool, tg = (pp_mm, "mm") if nh == 0 else (pp_sc, "sc")
                        bcm = pool.tile([P, 512], fp32, tag=tg, name="bcm")
                        mmr(bcm, ones_full[0:1, :], meanl[nh])
                        bcr = pool.tile([P, 512], fp32, tag=tg, name="bcr")
                        mmr(bcr, ones_full[0:1, :], rstdl[nh])
                        if copy_on_act:
                            nc.scalar.copy(bcm_sb[:, ssl], bcm)
                            nc.scalar.copy(bcr_sb[:, ssl], bcr)
                        else:
                            nc.vector.tensor_copy(bcm_sb[:, ssl], bcm)
                            nc.vector.tensor_copy(bcr_sb[:, ssl], bcr)
                    ln_norm(yl, g_t, be_t, bcm_sb, bcr_sb, out_tiles)

                layernorm(y, g1_t, be1_t)  # y is now mhaT

                # ---------------- FFN ---------------------------------------
                z = []
                for mt in range(NDT):
                    zt = xt_p.tile([P, S], bf16, tag="xt", name="zt",
                                   padded_shape=[P, 2 * S])
                    z.append(zt)
                for nh in range(NH):
                    ssl = slice(nh * 512, (nh + 1) * 512)
                    ff2_ps = []
                    for mt in range(NDT):
                        pool = pp_acc if mt < 2 else pp_sc
                        tag = "acc" if mt < 2 else "sc"
                        ff2_ps.append(
                            pool.tile([P, 512], fp32, tag=tag, name="ff2ps")
                        )
                    pending = None  # ft awaiting FFN2 emission
                    for ft in range(NFT):
                        ps = pp_mm.tile([P, 512], fp32, tag="mm", name="f1ps")
                        for dt in range(NDT):
                            mm(
                                ps,
                                w1_t[:, dt, ft, :],
                                y[dt][:, ssl],
                                start=(dt == 0),
                                stop=(dt == NDT - 1),
                            )
                        f1 = ff1_p.tile([P, 512], bf16, tag="ff1", name="f1")
                        if relu_on_act:
                            nc.scalar.activation(
                                f1, ps, AF.Relu, bias=b1_t[:, ft : ft + 1]
                            )
                        else:
                            nc.vector.tensor_scalar(
                                out=f1,
                                in0=ps,
                                scalar1=b1_t[:, ft : ft + 1],
                                scalar2=0.0,
                                op0=OP.add,
                                op1=OP.max,
                            )
                        if pending is not None:
                            pft, pf1 = pending
                            for mt in range(NDT):
                                mm(
                                    ff2_ps[mt],
                                    w2_t[:, pft, mt, :],
                                    pf1,
                                    start=(pft == 0),
                                    stop=False,
                                )
                        pending = (ft, f1)
                    pft, pf1 = pending
                    for mt in range(NDT):
                        mm(
                            ff2_ps[mt],
                            w2_t[:, pft, mt, :],
                            pf1,
                            start=False,
                            stop=True,
                        )
                        nc.vector.scalar_tensor_tensor(
                            z[mt][:, ssl],
                            ff2_ps[mt],
                            b2_t[:, mt : mt + 1],
                            y[mt][:, ssl],
                            OP.add,
                            OP.add,
                        )
                if l < L - 1:
                    layernorm(z, g2_t, be2_t)  # z is now next layer's xT
                    xt = z
                else:
                    ots = [
                        out_p.tile([P, S], fp32, tag="out", name="ot")
                        for _ in range(NDT)
                    ]
                    layernorm(z, g2_t, be2_t, out_tiles=ots)
                    for dt in range(NDT):
                        eng = nc.sync if dt % 2 == 0 else nc.scalar
                        eng.dma_start(out=out_d[dt], in_=ots[dt])

    return nc


def _prep_weights(Wq, Wk, Wv, Wo, ln1_g, ln1_b, W1, b1, W2, b2, ln2_g, ln2_b):
    f = np.float32
    bf = _bf16()

    def qk_r(W):  # [L,H,D,DK] -> [L, 128, NDT, NPAIR, 128]
        return np.ascontiguousarray(
            W.reshape(L, NPAIR, 2, NDT, P, DK)
            .transpose(0, 4, 3, 1, 2, 5)
            .reshape(L, P, NDT, NPAIR, P)
            .astype(bf)
        )

    wv_r = np.ascontiguousarray(
        Wv.transpose(0, 2, 1, 3)  # [L, D, H, DK]
        .reshape(L, NDT, P, H * DK)
        .transpose(0, 2, 1, 3)
        .reshape(L, P, NDT, H * DK)
        .astype(bf)
    )
    # Wo packed for K=128 pair-steps: [l, (head01, dk)=128, pr, mt, f]
    wo_r = np.ascontiguousarray(
        Wo.reshape(L, NPAIR, P, NDT, P).transpose(0, 2, 1, 3, 4).astype(bf)
    )
    w1_r = np.ascontiguousarray(
        W1.reshape(L, NDT, P, NFT, P).transpose(0, 2, 1, 3, 4).astype(bf)
    )
    w2_r = np.ascontiguousarray(
        W2.reshape(L, NFT, P, NDT, P).transpose(0, 2, 1, 3, 4).astype(bf)
    )

    def ln_r(v, n):  # [L, n*128] -> [L, 128, n]
        return np.ascontiguousarray(
            v.reshape(L, n, P).transpose(0, 2, 1).astype(f)
        )

    return {
        "wq": qk_r(Wq),
        "wk": qk_r(Wk),
        "wv": wv_r,
        "wo": wo_r,
        "w1": w1_r,
        "w2": w2_r,
        "g1": ln_r(ln1_g, NDT),
        "be1": ln_r(ln1_b, NDT),
        "g2": ln_r(ln2_g, NDT),
        "be2": ln_r(ln2_b, NDT),
        "b1": ln_r(b1, NFT),
        "b2": ln_r(b2, NDT),
    }


def get_nc():
    if "nc" not in _CACHE:
        nc = _build_nc()
        if not nc.is_finalized():
            nc.finalize()
        _CACHE["nc"] = nc
    return _CACHE["nc"]


def make_in_maps(**inputs):
    inputs = {k: np.asarray(v, dtype=np.float32) for k, v in inputs.items()}
    x = inputs.pop("x")
    wmap = _prep_weights(**inputs)
    in_maps = []
    wmap["ones"] = np.ones((P, P), dtype=np.float32)
    bf = _bf16()
    for b in range(B):
        xt = np.ascontiguousarray(x[b].T.reshape(NDT, P, S).astype(bf))
        in_maps.append({"x": xt, **wmap})
    return in_maps


def kernel(**inputs) -> np.ndarray:
    from concourse.bass_utils import run_bass_kernel_spmd

    nc = get_nc()
    in_maps = make_in_maps(**inputs)
    res = run_bass_kernel_spmd(nc, in_maps, core_ids=list(range(B)))
    out = np.empty((B, S, D), dtype=np.float32)
    for b in range(B):
        out[b] = res.results[b]["out"].reshape(D, S).T
    return out


if __name__ == "__main__":
    rng = np.random.default_rng(0)
    ins = {
        "x": rng.standard_normal((B, S, D), dtype=np.float32),
        "Wq": rng.standard_normal((L, H, D, DK), dtype=np.float32) * 0.02,
        "Wk": rng.standard_normal((L, H, D, DK), dtype=np.float32) * 0.02,
        "Wv": rng.standard_normal((L, H, D, DK), dtype=np.float32) * 0.02,
        "Wo": rng.standard_normal((L, D, D), dtype=np.float32) * 0.02,
        "ln1_g": np.ones((L, D), np.float32),
        "ln1_b": np.zeros((L, D), np.float32),
        "W1": rng.standard_normal((L, D, DFF), dtype=np.float32) * 0.02,
        "b1": np.zeros((L, DFF), np.float32),
        "W2": rng.standard_normal((L, DFF, D), dtype=np.float32) * 0.02,
        "b2": np.zeros((L, D), np.float32),
        "ln2_g": np.ones((L, D), np.float32),
        "ln2_b": np.zeros((L, D), np.float32),
    }
    out = kernel(**ins)
    print(out.shape, out.dtype, np.abs(out).mean())


# revision 37
# speedup vs baseline: 1.3383x; 1.3383x over previous
"""Trainium2 Bass kernel for a 6-layer post-LN transformer encoder.

Sharding: data-parallel over batch — B=8, one batch element per NeuronCore,
no collectives.  Each core runs the full 6-layer encoder on its [S, D] slice.

Device-side layout: activations are kept feature-major ([D, S], "xT") in SBUF
so that every matmul can use the natural input-major weights as the stationary
(lhsT) operand and PE contracts over the partition dim:

  out[m, n] = sum_k lhsT[k, m] * rhs[k, n]

Attention is computed transposed (scoresT[t, s]) so softmax needs no
transposes: denominators fall out of a ones-column in the ctx matmul, and the
per-column 1/denom broadcast is a k=1 matmul on PE.

v2 changes vs baseline:
  * bf16 activations + weights (fp32 PSUM accumulate).  2x DVE throughput on
    SBUF elementwise ops, half the DMA traffic.  Small stats rows stay fp32r.
  * Score matmuls for the two heads of a pair are emitted adjacently: K=64
    row-group tiling (auto tile_position (0,0)/(64,0)) runs them concurrently.
  * Both heads' scores land in one [P,1024] PSUM tile -> a single Exp
    activation per (pair, nh, t) halves ACT instruction overhead.
  * ctx for head B uses an M=128 stationary [ones|0(63)|V_B] so its rows land
    at partitions 64..127 (denominator at row 0): the per-pair ctx tile is
    [128, S] and Wo contracts K=128 (half the Wo matmuls).
  * Software-pipelined emission: scores(t+1) ahead of ctx(t), FFN1(ft) ahead
    of FFN2(ft-1), QK of pair p+1 between the two nh halves of pair p.
  * relu / PSUM->SBUF broadcast copies split between ScalarE and DVE.
"""

import numpy as np

L, H, D, DK, DFF = 6, 8, 512, 64, 2048
B, S = 8, 1024
EPS = 1e-5
P = 128
NDT = D // P        # 4  d-tiles
NST = S // P        # 8  s/t-tiles
NFT = DFF // P      # 16 dff-tiles
NPAIR = H // 2      # 4  head pairs
NH = S // 512       # 2  n-halves (512-wide fp32 matmul free dim)
FCH = 2             # W1 streamed in chunks of 2 dff-tiles
VBLK = 196          # per-pair V block: [V_A(64)|1|.|1|zeros|V_B(64) @130]
SCALE = 1.0 / np.sqrt(np.float32(DK))

_CACHE = {}


def _bf16():
    from concourse import mybir

    return mybir.dt.np(mybir.dt.bfloat16)


def _build_nc():
    import concourse.bass as bass
    import concourse.bacc as bacc
    import concourse.tile as tile
    from concourse import mybir

    fp32 = mybir.dt.float32
    fp32r = mybir.dt.float32r
    bf16 = mybir.dt.bfloat16
    AF = mybir.ActivationFunctionType
    OP = mybir.AluOpType

    class _Bacc(bacc.Bacc):
        # Exp (softmax) and Ln (layernorm rstd) live in different default
        # activation-table sets, causing ~50 table-load thrashes (~2.7us
        # each). Restrict both to natural_log_exp_and_others (which holds
        # both) so one load serves the whole kernel. Positional set ids are
        # preserved; only the function->set resolution changes.
        def insert_act_table_loads(self):
            from concourse.hw_specs import get_activation_tables
            import bass_rust as _bass_rust

            has_act = any(
                isinstance(i, mybir.InstActivation)
                for b in self.main_func.blocks
                for i in b.instructions
            )
            if not has_act:
                return
            AF2 = mybir.ActivationFunctionType
            tables = []
            for name, fns in get_activation_tables(self.m.arch).items():
                if name != "natural_log_exp_and_others":
                    fns = fns - {AF2.Exp, AF2.Ln}
                tables.append((name, fns))
            _bass_rust.insert_act_table_loads(self, tables)

    nc = _Bacc()

    from concourse.hw_specs import get_activation_tables

    _nl_set = get_activation_tables(nc.m.arch).get(
        "natural_log_exp_and_others", set()
    )
    relu_on_act = AF.Relu in _nl_set and AF.Copy in _nl_set
    copy_on_act = AF.Copy in _nl_set

    def mm(out, lhsT, rhs, **kw):
        return nc.tensor.matmul(out, lhsT, rhs, **kw)

    def mmr(out, lhsT, rhs, **kw):
        # fp32r matmul for the small stats/broadcast rows
        return nc.tensor.matmul(
            out, lhsT.bitcast(fp32r), rhs.bitcast(fp32r), **kw
        )

    def f(ap):
        # view a float32r tile as plain fp32 for DVE/ACT reads
        return ap.bitcast(fp32)

    x_d = nc.declare_dram_parameter("x", [NDT, P, S], bf16, isOutput=False)
    wq_d = nc.declare_dram_parameter(
        "wq", [L, P, NDT, NPAIR, P], bf16, isOutput=False
    )
    wk_d = nc.declare_dram_parameter(
        "wk", [L, P, NDT, NPAIR, P], bf16, isOutput=False
    )
    wv_d = nc.declare_dram_parameter("wv", [L, P, NDT, H * DK], bf16, isOutput=False)
    wo_d = nc.declare_dram_parameter(
        "wo", [L, P, NPAIR, NDT, P], bf16, isOutput=False
    )
    w1_d = nc.declare_dram_parameter(
        "w1", [L, P, NDT, NFT, P], bf16, isOutput=False
    )
    w2_d = nc.declare_dram_parameter(
        "w2", [L, P, NFT, NDT, P], bf16, isOutput=False
    )
    g1_d = nc.declare_dram_parameter("g1", [L, P, NDT], fp32, isOutput=False)
    be1_d = nc.declare_dram_parameter("be1", [L, P, NDT], fp32, isOutput=False)
    g2_d = nc.declare_dram_parameter("g2", [L, P, NDT], fp32, isOutput=False)
    be2_d = nc.declare_dram_parameter("be2", [L, P, NDT], fp32, isOutput=False)
    b1_d = nc.declare_dram_parameter("b1", [L, P, NFT], fp32, isOutput=False)
    b2_d = nc.declare_dram_parameter("b2", [L, P, NDT], fp32, isOutput=False)
    ones_d = nc.declare_dram_parameter("ones", [P, P], fp32r, isOutput=False)
    out_d = nc.declare_dram_parameter("out", [NDT, P, S], fp32, isOutput=True)

    with tile.TileContext(nc) as tc:
        from contextlib import ExitStack

        with ExitStack() as ctx:
            ec = ctx.enter_context
            ec(
                nc.allow_low_precision(
                    reason="bf16 matmul operands; fp32 PSUM accumulation"
                )
            )
            # --- SBUF pools ---
            const_p = ec(tc.tile_pool(name="const", bufs=1))
            wts_p = ec(tc.tile_pool(name="wts", bufs=2))
            xt_p = ec(tc.tile_pool(name="xt", bufs=4))
            qk_p = ec(tc.tile_pool(name="qk", bufs=2))
            v_p = ec(tc.tile_pool(name="v", bufs=8))
            exp_p = ec(tc.tile_pool(name="exp", bufs=4))
            ctx_p = ec(tc.tile_pool(name="ctxp", bufs=4))
            mha_p = ec(tc.tile_pool(name="mha", bufs=4))
            ff1_p = ec(tc.tile_pool(name="ff1", bufs=4))
            ysq_p = ec(tc.tile_pool(name="ysq", bufs=2))
            bcs_p = ec(tc.tile_pool(name="bcs", bufs=2))
            rows_p = ec(tc.tile_pool(name="rows", bufs=1))
            out_p = ec(tc.tile_pool(name="outp", bufs=4))
            # --- PSUM pools: 4 + 2 + 2 = 8 banks ---
            # sc:  [P,1024] scores (A|B) tiles; FFN borrows [P,512] slots
            # acc: attention ctx accumulators (A, B)
            # mm:  short-lived matmul outputs (QKV/V/Wo/FFN1/LN stats+bc)
            pp_sc = ec(tc.tile_pool(name="pp_sc", bufs=2, space="PSUM"))
            pp_acc = ec(tc.tile_pool(name="pp_acc", bufs=2, space="PSUM"))
            pp_mm = ec(tc.tile_pool(name="pp_mm", bufs=2, space="PSUM"))

            # ones come from DRAM so the fp32r data counts as pre-rounded
            ones_full = const_p.tile([P, P], fp32r)
            nc.sync.dma_start(out=ones_full, in_=ones_d[:, :])
            ones_col_bf = const_p.tile([P, 1], bf16)
            nc.vector.memset(ones_col_bf, 1.0)
            # selector rows for the paired 1/denom broadcast:
            #   selA row: [1]*64 + [0]*64   selB row: [0]*64 + [1]*64
            selA_t = const_p.tile([P, P], fp32r)
            nc.vector.memset(f(selA_t)[:, 0:64], 1.0)
            nc.vector.memset(f(selA_t)[:, 64:128], 0.0)
            selB_t = const_p.tile([P, P], fp32r)
            nc.vector.memset(f(selB_t)[:, 0:64], 0.0)
            nc.vector.memset(f(selB_t)[:, 64:128], 1.0)
            zero_col = const_p.tile([P, 1], fp32)
            nc.vector.memset(zero_col, 0.0)
            eps_col = const_p.tile([P, 1], fp32)
            nc.vector.memset(eps_col, float(EPS))

            # layer-0 input
            xt = []
            for dt in range(NDT):
                t = xt_p.tile([P, S], bf16, tag="xt")
                (nc.sync if dt % 2 == 0 else nc.scalar).dma_start(
                    out=t, in_=x_d[dt]
                )
                xt.append(t)

            def make_qk(pr, w_t, tag):
                # both nh halves' matmuls adjacent per dt: the second reuses
                # the stationary (no LDWEIGHTS reload on HW)
                dst = qk_p.tile([P, S], bf16, tag=tag, name="qkdst")
                ps0 = pp_mm.tile([P, 512], fp32, tag="mm", name="qkps0")
                ps1 = pp_mm.tile([P, 512], fp32, tag="mm", name="qkps1")
                for dt in range(NDT):
                    for nh, ps in ((0, ps0), (1, ps1)):
                        mm(
                            ps,
                            w_t[:, dt, pr, :],
                            xt[dt][:, nh * 512 : (nh + 1) * 512],
                            start=(dt == 0),
                            stop=(dt == NDT - 1),
                        )
                nc.vector.tensor_copy(dst[:, 0:512], ps0)
                nc.vector.tensor_copy(dst[:, 512:1024], ps1)
                return dst

            for l in range(L):
                # ---------------- weight loads (bufs=2 pools: next layer's
                # loads overlap this layer's compute) ------------------------
                wq_t = wts_p.tile([P, NDT, NPAIR, P], bf16, tag="wq")
                nc.sync.dma_start(out=wq_t, in_=wq_d[l])
                wk_t = wts_p.tile([P, NDT, NPAIR, P], bf16, tag="wk")
                nc.sync.dma_start(out=wk_t, in_=wk_d[l])
                wv_t = wts_p.tile([P, NDT, H * DK], bf16, tag="wv")
                nc.sync.dma_start(out=wv_t, in_=wv_d[l])
                wo_t = wts_p.tile([P, NPAIR, NDT, P], bf16, tag="wo")
                nc.sync.dma_start(out=wo_t, in_=wo_d[l])
                g1_t = wts_p.tile([P, NDT], fp32, tag="g1")
                nc.sync.dma_start(out=g1_t, in_=g1_d[l])
                be1_t = wts_p.tile([P, NDT], fp32, tag="be1")
                nc.sync.dma_start(out=be1_t, in_=be1_d[l])
                g2_t = wts_p.tile([P, NDT], fp32, tag="g2")
                nc.sync.dma_start(out=g2_t, in_=g2_d[l])
                be2_t = wts_p.tile([P, NDT], fp32, tag="be2")
                nc.sync.dma_start(out=be2_t, in_=be2_d[l])
                b1_t = wts_p.tile([P, NFT], fp32, tag="b1")
                nc.sync.dma_start(out=b1_t, in_=b1_d[l])
                b2_t = wts_p.tile([P, NDT], fp32, tag="b2")
                nc.sync.dma_start(out=b2_t, in_=b2_d[l])
                w1_t = wts_p.tile([P, NDT, NFT, P], bf16, tag="w1")
                nc.sync.dma_start(out=w1_t, in_=w1_d[l])
                w2_t = wts_p.tile([P, NFT, NDT, P], bf16, tag="w2")
                nc.sync.dma_start(out=w2_t, in_=w2_d[l])

                # ---------------- Q/K for pair 0 ----------------------------
                qt = [None] * NPAIR
                kt = [None] * NPAIR
                qt[0] = make_qk(0, wq_t, "qt")
                kt[0] = make_qk(0, wk_t, "kt")

                # ---------------- V = x @ Wv, packed per head pair ----------
                # vt[:, pr, 0:64]    = V of head 2*pr       (ctx rows 0..63)
                # vt[:, pr, 64]      = 1                    (denom A, row 64)
                # vt[:, pr, 66]      = 1                    (denom B, row 0)
                # vt[:, pr, 67:130]  = 0                    (junk rows 1..63)
                # vt[:, pr, 130:194] = V of head 2*pr+1     (ctx rows 64..127)
                v_tiles = [None] * NST

                def make_v(st):
                    vt = v_p.tile([P, NPAIR, VBLK], bf16, tag="v", name="vt")
                    ps = pp_mm.tile([P, 512], fp32, tag="mm", name="vps")
                    for dt in range(NDT):
                        mm(
                            ps,
                            xt[dt][:, st * P : (st + 1) * P],
                            wv_t[:, dt, :],
                            start=(dt == 0),
                            stop=(dt == NDT - 1),
                        )
                    psh = ps.rearrange("p (h k) -> p h k", h=H)
                    nc.vector.tensor_copy(vt[:, :, 0:DK], psh[:, 0::2, :])
                    nc.vector.tensor_copy(vt[:, :, 130 : 130 + DK], psh[:, 1::2, :])
                    nc.gpsimd.memset(vt[:, :, 67:130], 0.0)
                    nc.gpsimd.memset(vt[:, :, 64:67], 1.0)
                    v_tiles[st] = vt

                # V[0..1] up front; the rest fill just-in-time inside the
                # first pair's t-loop so exp work starts sooner
                make_v(0)
                make_v(1)

                # ---------------- attention ---------------------------------
                ctx_tiles = []
                for pr in range(NPAIR):
                    ch = ctx_p.tile([P, S], bf16, tag="ctx", name="ch")
                    ctx_tiles.append(ch)

                def scores(pr, nh, t):
                    ssl = slice(nh * 512, (nh + 1) * 512)
                    tsl = slice(t * P, (t + 1) * P)
                    sc = pp_sc.tile([P, 1024], fp32, tag="sc", name="sc")
                    # two K=64 matmuls on distinct row groups -> concurrent
                    mm(sc[:, 0:512], kt[pr][0:64, tsl], qt[pr][0:64, ssl])
                    mm(sc[:, 512:1024], kt[pr][64:128, tsl], qt[pr][64:128, ssl])
                    return sc

                def attend(pr, nh, vfill=False):
                    ssl = slice(nh * 512, (nh + 1) * 512)
                    psA = pp_acc.tile([P, 512], fp32, tag="acc", name="psA")
                    psB = pp_acc.tile([P, 512], fp32, tag="acc", name="psB")
                    sc_cur = scores(pr, nh, 0)
                    for t in range(NST):
                        if vfill and t + 2 < NST:
                            make_v(t + 2)
                        sc_next = scores(pr, nh, t + 1) if t + 1 < NST else None
                        e = exp_p.tile([P, 1024], bf16, tag="exp", name="e")
                        nc.scalar.activation(
                            e, sc_cur, AF.Exp, bias=zero_col, scale=float(SCALE)
                        )
                        vt = v_tiles[t]
                        mm(
                            psA[0:65, :],
                            vt[:, pr, 0 : DK + 1],
                            e[:, 0:512],
                            start=(t == 0),
                            stop=(t == NST - 1),
                        )
                        mm(
                            psB,
                            vt[:, pr, 66:194],
                            e[:, 512:1024],
                            start=(t == 0),
                            stop=(t == NST - 1),
                        )
                        sc_cur = sc_next
                    # normalize: ctx rows / denominator (A: row 64, B: row 0).
                    # ScalarE copies the raw ctx rows out first so the acc
                    # PSUM banks release early (next pair's ctx can start);
                    # the 1/denom scale then runs in-place in SBUF with the
                    # broadcast read straight from PSUM.
                    rA = rows_p.tile([65, 512], fp32r, tag="rA", bufs=2, name="rA")
                    nc.vector.reciprocal(rA[64:65], psA[64:65])
                    rB = rows_p.tile([1, 512], fp32r, tag="rB", bufs=2, name="rB")
                    nc.vector.reciprocal(rB, psB[0:1])
                    ch = ctx_tiles[pr]
                    nc.vector.tensor_copy(ch[0:64, ssl], psA[0:64])
                    nc.vector.tensor_copy(ch[64:128, ssl], psB[64:128])
                    # bc rows 0..63 = 1/dA, rows 64..127 = 1/dB via selector
                    # rows (two accumulating M=128 matmuls, no col tiling)
                    bc = pp_mm.tile([P, 512], fp32, tag="mm", name="bc")
                    mmr(bc, selA_t[64:65, :], rA[64:65], start=True, stop=False)
                    mmr(bc, selB_t[0:1, :], rB, start=False, stop=True)
                    nc.vector.tensor_mul(ch[0:64, ssl], ch[0:64, ssl], bc[0:64])
                    nc.vector.tensor_mul(ch[64:128, ssl], ch[64:128, ssl], bc[64:128])

                for pr in range(NPAIR):
                    attend(pr, 0, vfill=(pr == 0))
                    if pr + 1 < NPAIR:
                        # PE work to cover the normalize tail / acc release
                        qt[pr + 1] = make_qk(pr + 1, wq_t, "qt")
                        kt[pr + 1] = make_qk(pr + 1, wk_t, "kt")
                    attend(pr, 1)

                # ---------------- Wo + residual -> y (pre-LN1) --------------
                y = []
                for mt in range(NDT):
                    yt = mha_p.tile([P, S], bf16, tag="mha", name="yt")
                    y.append(yt)
                for mt in range(NDT):
                    ps0 = pp_mm.tile([P, 512], fp32, tag="mm", name="wops0")
                    ps1 = pp_mm.tile([P, 512], fp32, tag="mm", name="wops1")
                    for pr in range(NPAIR):
                        for nh, ps in ((0, ps0), (1, ps1)):
                            mm(
                                ps,
                                wo_t[:, pr, mt, :],
                                ctx_tiles[pr][:, nh * 512 : (nh + 1) * 512],
                                start=(pr == 0),
                                stop=(pr == NPAIR - 1),
                            )
                    nc.vector.tensor_add(y[mt][:, 0:512], ps0, xt[mt][:, 0:512])
                    nc.vector.tensor_add(y[mt][:, 512:1024], ps1, xt[mt][:, 512:1024])

                def ln_stats(yl, nh, bcm_sb, bcr_sb):
                    # per-half LN stats chain: ones-matmul sums -> mean/var
                    # rows -> rstd via Ln/Exp -> k=1 broadcast matmuls ->
                    # SBUF copies into the given [P,1024] broadcast tiles
                    ssl = slice(nh * 512, (nh + 1) * 512)
                    pool, tg = (pp_mm, "mm") if nh == 0 else (pp_sc, "sc")
                    p1 = pool.tile([1, 512], fp32, tag=tg, name="p1")
                    for dt in range(NDT):
                        mm(
                            p1,
                            ones_col_bf,
                            yl[dt][:, ssl],
                            start=(dt == 0),
                            stop=(dt == NDT - 1),
                        )
                    p2 = pool.tile([1, 512], fp32, tag=tg, name="p2")
                    for dt in range(NDT):
                        sq = ysq_p.tile([P, 512], bf16, tag="ysq", name="sq")
                        nc.vector.tensor_mul(sq, yl[dt][:, ssl], yl[dt][:, ssl])
                        mm(
                            p2,
                            ones_col_bf,
                            sq,
                            start=(dt == 0),
                            stop=(dt == NDT - 1),
                        )
                    mean = rows_p.tile(
                        [1, 512], fp32r, tag=f"mean{nh}", bufs=1, name="mean"
                    )
                    nc.vector.tensor_scalar_mul(mean, p1, 1.0 / D)
                    # mean^2 on ScalarE (Square with scale=1/D), in parallel
                    # with DVE computing the mean row
                    var = rows_p.tile(
                        [1, 512], fp32, tag=f"var{nh}", bufs=1, name="var"
                    )
                    if copy_on_act:
                        nc.scalar.activation(var, p1, AF.Square, scale=1.0 / D)
                    else:
                        nc.vector.tensor_mul(var, f(mean), f(mean))
                    nc.vector.scalar_tensor_tensor(
                        var, p2, 1.0 / D, var, OP.mult, OP.subtract
                    )
                    # rstd = exp(-0.5 * ln(var + eps))
                    nc.scalar.activation(var, var, AF.Ln, bias=eps_col[0:1])
                    rstd = rows_p.tile(
                        [1, 512], fp32r, tag=f"rstd{nh}", bufs=1, name="rstd"
                    )
                    nc.scalar.activation(
                        rstd, var, AF.Exp, bias=zero_col[0:1], scale=-0.5
                    )
                    bcm = pool.tile([P, 512], fp32, tag=tg, name="bcm")
                    mmr(bcm, ones_full[0:1, :], mean)
                    bcr = pool.tile([P, 512], fp32, tag=tg, name="bcr")
                    mmr(bcr, ones_full[0:1, :], rstd)
                    if copy_on_act:
                        nc.scalar.copy(bcm_sb[:, ssl], bcm)
                        nc.scalar.copy(bcr_sb[:, ssl], bcr)
                    else:
                        nc.vector.tensor_copy(bcm_sb[:, ssl], bcm)
                        nc.vector.tensor_copy(bcr_sb[:, ssl], bcr)

                def ln_norm(yl, g_t, be_t, bcm_sb, bcr_sb, out_tiles=None):
                    for dt in range(NDT):
                        for nh in range(NH):
                            ssl = slice(nh * 512, (nh + 1) * 512)
                            yr = yl[dt][:, ssl]
                            dst = (
                                yr
                                if out_tiles is None
                                else out_tiles[dt][:, ssl]
                            )
                            eng = nc.gpsimd if dt >= 2 else nc.vector
                            eng.tensor_sub(yr, yr, bcm_sb[:, ssl])
                            eng.tensor_mul(yr, yr, bcr_sb[:, ssl])
                            eng.tensor_scalar(
                                out=dst,
                                in0=yr,
                                scalar1=g_t[:, dt : dt + 1],
                                scalar2=be_t[:, dt : dt + 1],
                                op0=OP.mult,
                                op1=OP.add,
                            )

                def layernorm(yl, g_t, be_t, out_tiles=None):
                    # the two nh-half chains are interleaved stage by stage so
                    # DVE/ACT latency pipelines across halves
                    bcm_sb = bcs_p.tile([P, 1024], bf16, tag="bcs", name="bcmsb")
                    bcr_sb = bcs_p.tile([P, 1024], bf16, tag="bcs", name="bcrsb")
                    p1l, p2l, meanl, varl, rstdl = [], [], [], [], []
                    for nh in range(NH):
                        ssl = slice(nh * 512, (nh + 1) * 512)
                        pool, tg = (pp_mm, "mm") if nh == 0 else (pp_sc, "sc")
                        p1 = pool.tile([1, 512], fp32, tag=tg, name="p1")
                        for dt in range(NDT):
                            mm(
                                p1,
                                ones_col_bf,
                                yl[dt][:, ssl],
                                start=(dt == 0),
                                stop=(dt == NDT - 1),
                            )
                        p2 = pool.tile([1, 512], fp32, tag=tg, name="p2")
                        for dt in range(NDT):
                            sq = ysq_p.tile([P, 512], bf16, tag="ysq", name="sq")
                            nc.vector.tensor_mul(
                                sq, yl[dt][:, ssl], yl[dt][:, ssl]
                            )
                            mm(
                                p2,
                                ones_col_bf,
                                sq,
                                start=(dt == 0),
                                stop=(dt == NDT - 1),
                            )
                        p1l.append(p1)
                        p2l.append(p2)
                    for nh in range(NH):
                        mean = rows_p.tile(
                            [1, 512], fp32r, tag=f"mean{nh}", bufs=1, name="mean"
                        )
                        nc.vector.tensor_scalar_mul(mean, p1l[nh], 1.0 / D)
                        meanl.append(mean)
                    for nh in range(NH):
                        # mean^2 on ScalarE (Square with scale=1/D), parallel
                        # with the DVE mean rows
                        var = rows_p.tile(
                            [1, 512], fp32, tag=f"var{nh}", bufs=1, name="var"
                        )
                        if copy_on_act:
                            nc.scalar.activation(
                                var, p1l[nh], AF.Square, scale=1.0 / D
                            )
                        else:
                            nc.vector.tensor_mul(var, f(meanl[nh]), f(meanl[nh]))
                        nc.vector.scalar_tensor_tensor(
                            var, p2l[nh], 1.0 / D, var, OP.mult, OP.subtract
                        )
                        varl.append(var)
                    for nh in range(NH):
                        nc.scalar.activation(
                            varl[nh], varl[nh], AF.Ln, bias=eps_col[0:1]
                        )
                    for nh in range(NH):
                        rstd = rows_p.tile(
                            [1, 512], fp32r, tag=f"rstd{nh}", bufs=1, name="rstd"
                        )
                        nc.scalar.activation(
                            rstd, varl[nh], AF.Exp, bias=zero_col[0:1], scale=-0.5
                        )
                        rstdl.append(rstd)
                    for nh in range(NH):
                        ssl = slice(nh * 512, (nh + 1) * 512)
                        pool, tg = (pp_mm, "mm") if nh == 0 else (pp_sc, "sc")
                        bcm = pool.tile([P, 512], fp32, tag=tg, name="bcm")
                        mmr(bcm, ones_full[0:1, :], meanl[nh])
                        bcr = pool.tile([P, 512], fp32, tag=tg, name="bcr")
                        mmr(bcr, ones_full[0:1, :], rstdl[nh])
                        if copy_on_act:
                            nc.scalar.copy(bcm_sb[:, ssl], bcm)
                            nc.scalar.copy(bcr_sb[:, ssl], bcr)
                        else:
                            nc.vector.tensor_copy(bcm_sb[:, ssl], bcm)
                            nc.vector.tensor_copy(bcr_sb[:, ssl], bcr)
                    ln_norm(yl, g_t, be_t, bcm_sb, bcr_sb, out_tiles)

                layernorm(y, g1_t, be1_t)  # y is now mhaT

                # ---------------- FFN ---------------------------------------
                z = []
                for mt in range(NDT):
                    zt = xt_p.tile([P, S], bf16, tag="xt", name="zt",
                                   padded_shape=[P, 2 * S])
                    z.append(zt)
                for nh in range(NH):
                    ssl = slice(nh * 512, (nh + 1) * 512)
                    ff2_ps = []
                    for mt in range(NDT):
                        pool = pp_acc if mt < 2 else pp_sc
                        tag = "acc" if mt < 2 else "sc"
                        ff2_ps.append(
                            pool.tile([P, 512], fp32, tag=tag, name="ff2ps")
                        )
                    pending = None  # ft awaiting FFN2 emission
                    for ft in range(NFT):
                        ps = pp_mm.tile([P, 512], fp32, tag="mm", name="f1ps")
                        for dt in range(NDT):
                            mm(
                                ps,
                                w1_t[:, dt, ft, :],
                                y[dt][:, ssl],
                                start=(dt == 0),
                                stop=(dt == NDT - 1),
                            )
                        f1 = ff1_p.tile([P, 512], bf16, tag="ff1", name="f1")
                        if relu_on_act:
                            nc.scalar.activation(
                                f1, ps, AF.Relu, bias=b1_t[:, ft : ft + 1]
                            )
                        else:
                            nc.vector.tensor_scalar(
                                out=f1,
                                in0=ps,
                                scalar1=b1_t[:, ft : ft + 1],
                                scalar2=0.0,
                                op0=OP.add,
                                op1=OP.max,
                            )
                        if pending is not None:
                            pft, pf1 = pending
                            for mt in range(NDT):
                                mm(
                                    ff2_ps[mt],
                                    w2_t[:, pft, mt, :],
                                    pf1,
                                    start=(pft == 0),
                                    stop=False,
                                )
                        pending = (ft, f1)
                    pft, pf1 = pending
                    for mt in range(NDT):
                        mm(
                            ff2_ps[mt],
                            w2_t[:, pft, mt, :],
                            pf1,
                            start=False,
                            stop=True,
                        )
                        nc.vector.scalar_tensor_tensor(
                            z[mt][:, ssl],
                            ff2_ps[mt],
                            b2_t[:, mt : mt + 1],
                            y[mt][:, ssl],
                            OP.add,
                            OP.add,
                        )
                if l < L - 1:
                    layernorm(z, g2_t, be2_t)  # z is now next layer's xT
                    xt = z
                else:
                    ots = [
                        out_p.tile([P, S], fp32, tag="out", name="ot")
                        for _ in range(NDT)
                    ]
                    layernorm(z, g2_t, be2_t, out_tiles=ots)
                    for dt in range(NDT):
                        for nh in range(NH):
                            ssl = slice(nh * 512, (nh + 1) * 512)
                            eng = nc.sync if (dt + nh) % 2 == 0 else nc.scalar
                            eng.dma_start(
                                out=out_d[dt][:, ssl], in_=ots[dt][:, ssl]
                            )

    return nc


def _prep_weights(Wq, Wk, Wv, Wo, ln1_g, ln1_b, W1, b1, W2, b2, ln2_g, ln2_b):
    f = np.float32
    bf = _bf16()

    def qk_r(W):  # [L,H,D,DK] -> [L, 128, NDT, NPAIR, 128]
        return np.ascontiguousarray(
            W.reshape(L, NPAIR, 2, NDT, P, DK)
            .transpose(0, 4, 3, 1, 2, 5)
            .reshape(L, P, NDT, NPAIR, P)
            .astype(bf)
        )

    wv_r = np.ascontiguousarray(
        Wv.transpose(0, 2, 1, 3)  # [L, D, H, DK]
        .reshape(L, NDT, P, H * DK)
        .transpose(0, 2, 1, 3)
        .reshape(L, P, NDT, H * DK)
        .astype(bf)
    )
    # Wo packed for K=128 pair-steps: [l, (head01, dk)=128, pr, mt, f]
    wo_r = np.ascontiguousarray(
        Wo.reshape(L, NPAIR, P, NDT, P).transpose(0, 2, 1, 3, 4).astype(bf)
    )
    w1_r = np.ascontiguousarray(
        W1.reshape(L, NDT, P, NFT, P).transpose(0, 2, 1, 3, 4).astype(bf)
    )
    w2_r = np.ascontiguousarray(
        W2.reshape(L, NFT, P, NDT, P).transpose(0, 2, 1, 3, 4).astype(bf)
    )

    def ln_r(v, n):  # [L, n*128] -> [L, 128, n]
        return np.ascontiguousarray(
            v.reshape(L, n, P).transpose(0, 2, 1).astype(f)
        )

    return {
        "wq": qk_r(Wq),
        "wk": qk_r(Wk),
        "wv": wv_r,
        "wo": wo_r,
        "w1": w1_r,
        "w2": w2_r,
        "g1": ln_r(ln1_g, NDT),
        "be1": ln_r(ln1_b, NDT),
        "g2": ln_r(ln2_g, NDT),
        "be2": ln_r(ln2_b, NDT),
        "b1": ln_r(b1, NFT),
        "b2": ln_r(b2, NDT),
    }


def get_nc():
    if "nc" not in _CACHE:
        nc = _build_nc()
        if not nc.is_finalized():
            nc.finalize()
        _CACHE["nc"] = nc
    return _CACHE["nc"]


def make_in_maps(**inputs):
    inputs = {k: np.asarray(v, dtype=np.float32) for k, v in inputs.items()}
    x = inputs.pop("x")
    wmap = _prep_weights(**inputs)
    in_maps = []
    wmap["ones"] = np.ones((P, P), dtype=np.float32)
    bf = _bf16()
    for b in range(B):
        xt = np.ascontiguousarray(x[b].T.reshape(NDT, P, S).astype(bf))
        in_maps.append({"x": xt, **wmap})
    return in_maps


def kernel(**inputs) -> np.ndarray:
    from concourse.bass_utils import run_bass_kernel_spmd

    nc = get_nc()
    in_maps = make_in_maps(**inputs)
    res = run_bass_kernel_spmd(nc, in_maps, core_ids=list(range(B)))
    out = np.empty((B, S, D), dtype=np.float32)
    for b in range(B):
        out[b] = res.results[b]["out"].reshape(D, S).T
    return out


if __name__ == "__main__":
    rng = np.random.default_rng(0)
    ins = {
        "x": rng.standard_normal((B, S, D), dtype=np.float32),
        "Wq": rng.standard_normal((L, H, D, DK), dtype=np.float32) * 0.02,
        "Wk": rng.standard_normal((L, H, D, DK), dtype=np.float32) * 0.02,
        "Wv": rng.standard_normal((L, H, D, DK), dtype=np.float32) * 0.02,
        "Wo": rng.standard_normal((L, D, D), dtype=np.float32) * 0.02,
        "ln1_g": np.ones((L, D), np.float32),
        "ln1_b": np.zeros((L, D), np.float32),
        "W1": rng.standard_normal((L, D, DFF), dtype=np.float32) * 0.02,
        "b1": np.zeros((L, DFF), np.float32),
        "W2": rng.standard_normal((L, DFF, D), dtype=np.float32) * 0.02,
        "b2": np.zeros((L, D), np.float32),
        "ln2_g": np.ones((L, D), np.float32),
        "ln2_b": np.zeros((L, D), np.float32),
    }
    out = kernel(**ins)
    print(out.shape, out.dtype, np.abs(out).mean())
